# revision 20
# baseline (speedup 1.0000x reference)
"""GroupNorm + single-head self-attention block (B=16, C=512, H=W=32) on 8
TRN2 NeuronCores.

Sharding: pure data-parallel over batch - 2 samples per core, no collectives.

Algebraic restructure vs the straightforward q/k/v pipeline (exact, done on
the host in f64):

  scores  = (Wq h)^T (Wk h) = h^T M h        with M  = Wq^T Wk
  o       = attn @ (V Wo^T)                  with WB = Wo @ Wv  (attn weights
            are scalars, so the output projection commutes into V)

so the device only runs TWO projection matmuls (G = M h and Vb = WB h)
instead of four (q, k, v, o2): 25% fewer PE columns and ~30% fewer PSUM
evacuations.  Biases stay exact:
  - bv, bo enter as c0 = Wo bv + bo, a per-channel constant added at the
    final (channel-major) evacuation;
  - bq, bk survive softmax only through the per-key term z[m] = (Wk^T bq) .
    h[:, m]; z is computed as one extra moving column riding the Vb matmul
    (whose output is pixel-major, so z lands per-partition) and folded into
    the exp bias. Per-query terms cancel in softmax.

Matmuls run in fp8-e4m3 with perf_mode=DoubleRow (2 contraction slices per
pass), weights pre-scaled x16 into the e4m3 sweet spot, exp shifted by -1.5
so E stays far below the TRN fp8 max of 240. fp32 PSUM accumulation.
Numpy-prototyped rel err vs the fp32 reference: 4.3e-3 (budget 2e-2).

Per-sample dataflow (C=512 channels, N=1024 pixels), channels-on-partitions:

  x    [128, CT=4, N]   f32
  GN:  bn_stats/bn_aggr per channel, group (16-ch) aggregation via a tiny
       matmul against a group-indicator matrix; h = a_c*x - b_c -> fp8
  G    [128, CT, N] = (M.T tiles).T @ h        (DR pairs over c')
  Vb   [128, MT=8, C] = h_tile.T @ WB.T tiles  (pixel-major; z column rides)
  ST   [m, n] = G_tile.T @ h                   (scores, transposed layout)
  E    = exp(ST*ISC + bias)  -> fp8            (bias = z/sqrt(C) - SHIFT)
  R    = 1/(16.T ones @ E)                     (softmax denominators, PE)
  OT   [c, n] = Vb_tile.T @ E                  (un-normalized attn output)
  y    = (OT*R + c0) + x                       (DVE mul in psum, Pool stt)
"""

import numpy as np

import concourse.bass as bass
import concourse.mybir as mybir
from concourse import tile
from concourse.bass_utils import run_bass_kernel_spmd


def _install_drain_patch():
    """This walrus build rejects Drain instructions carrying more than one
    semaphore wait (setupSyncWait<CTRL_NO_STRUCT>). Split the TileContext
    tail drain's waits across a chain of single-wait drains."""
    import concourse.tile as tile_mod
    from concourse.vector_clock import ScopedClock

    if getattr(tile_mod.TileContext, "_drain_patch_installed", False):
        return

    def _patched(self, tick_clock, wait_clock):
        nc = self.nc
        drain_inst = nc.sync.drain()
        wait_clock.add_sem_waits(
            drain_inst.ins, ScopedClock({None: tick_clock.global_clock})
        )
        si = drain_inst.ins.sync_info
        waits = list(si.on_wait or []) if si is not None else []
        if len(waits) > 1:
            si.on_wait = waits[:1]
            for w in waits[1:]:
                extra = nc.sync.drain()
                extra.ins.sync_info = mybir.SyncInfo(on_wait=[w], on_update=[])

        nc.all_engine_barrier()
        assert self.sems is not None
        popped = nc._tile_sem_poison_stack.pop()
        assert popped is self._sem_poison
        nc.clear_and_free_semaphores(list(self.sems.allocated().values()))
        nc.all_engine_barrier()

    tile_mod.TileContext._drain_and_barrier = _patched
    tile_mod.TileContext._drain_patch_installed = True


_install_drain_patch()

F32 = mybir.dt.float32
BF16 = mybir.dt.bfloat16
FP8 = mybir.dt.float8e4

B, C, H, W = 16, 512, 32, 32
N = H * W                      # 1024 pixels
NCORES = 8
S = B // NCORES                # samples per core
CT = C // 128                  # 4 channel tiles
MT = N // 128                  # 8 pixel tiles
NW = 512                       # matmul output chunk (PSUM bank = 512 f32)
NCH = N // NW                  # 2 chunks
GROUPS = 32
GSIZE = C // GROUPS            # 16 channels per group
GPT = 128 // GSIZE             # 8 groups per channel tile
EPS = 1e-5

COMPUTE = "fp8"                # "fp8" (DoubleRow) | "bf16"

CDT = {"fp8": FP8, "bf16": BF16}[COMPUTE]
SCL = 16.0 if COMPUTE == "fp8" else 1.0
SHIFT = 1.5
ISC = 1.0 / (SCL * float(np.sqrt(C)))
DR = mybir.MatmulPerfMode.DoubleRow if COMPUTE == "fp8" else None
KSTEP = 2 if COMPUTE == "fp8" else 1   # contraction subtiles per matmul


def _split_waits(nc, maxw=1):
    """This walrus build caps the number of sync waits an instruction can
    carry (varies by instruction class; Drain and Matmult/LDWEIGHTS observed
    failing). Hoist excess waits onto standalone EventSemaphore instructions
    inserted just before, on the same engine."""
    cnt = 0
    for f in nc.m.functions:
        for bb in f.blocks:
            insts = list(bb.instructions)
            out = []
            changed = False
            for inst in insts:
                si = inst.sync_info
                waits = list(si.on_wait) if (si is not None and si.on_wait) else []
                if len(waits) > maxw:
                    for w in waits[:-maxw]:
                        ev = mybir.InstEventSemaphore(
                            name=f"waitsplit_{cnt}", ins=[], outs=[])
                        cnt += 1
                        ev.engine = inst.engine
                        ev.sync_info = mybir.SyncInfo(on_wait=[w], on_update=[])
                        out.append(ev)
                    si.on_wait = waits[-maxw:]
                    changed = True
                out.append(inst)
            if changed:
                _replace_block_instructions(bb, out)
    return cnt


def _replace_block_instructions(bb, insts):
    try:
        bb.instructions = insts
        return
    except Exception:
        pass
    try:
        bb.instructions.clear()
        for i in insts:
            bb.instructions.append(i)
        return
    except Exception:
        pass
    raise RuntimeError("cannot rewrite block instructions")


def build_nc(split_waits=True):
    nc = bass.Bass(target_bir_lowering=False)

    x_ext = nc.declare_dram_parameter("x", [S, CT, 128, N], F32, isOutput=False)
    m8_ext = nc.declare_dram_parameter("m8t", [128, CT, C], CDT, isOutput=False)
    # wbt carries WB.T in cols 0:512 and the z-vector u in col 512 (+pad)
    wb_ext = nc.declare_dram_parameter("wbt8", [128, CT, C + 16], CDT,
                                       isOutput=False)
    # cblob columns: gnw[4] gnb[4] c0[4] gmat[8] -> [128, 20] f32
    cb_ext = nc.declare_dram_parameter("cblob", [128, 20], F32, isOutput=False)
    gmt_ext = nc.declare_dram_parameter("gmt", [GPT, 128], F32, isOutput=False)
    ones_ext = nc.declare_dram_parameter("ones16", [128, 2, 128], CDT,
                                         isOutput=False)
    out_ext = nc.declare_dram_parameter("out", [S, CT, 128, N], F32,
                                        isOutput=True)

    with tile.TileContext(nc) as tc:
        _body(nc, tc, x_ext, m8_ext, wb_ext, cb_ext, gmt_ext, ones_ext,
              out_ext)
    if split_waits:
        _split_waits(nc)
    return nc


def _mm_accum(nc, psum, lhsT3, rhs3, kparts):
    """Accumulating matmul over contraction subtiles. lhsT3/rhs3 are
    callables j -> AP: for fp8 they get slice(j, j+2) (DoubleRow pair),
    for bf16 slice(j, j+1)."""
    steps = list(range(0, kparts, KSTEP))
    for i, j in enumerate(steps):
        nc.tensor.matmul(
            psum,
            lhsT3(slice(j, j + KSTEP)),
            rhs3(slice(j, j + KSTEP)),
            start=(i == 0), stop=(i == len(steps) - 1),
            perf_mode=DR,
        )


def _body(nc, tc, x_ext, m8_ext, wb_ext, cb_ext, gmt_ext, ones_ext, out_ext):
    import contextlib

    ctx = contextlib.ExitStack()
    with ctx:
        consts = ctx.enter_context(tc.tile_pool(name="consts", bufs=1))
        sb = ctx.enter_context(tc.tile_pool(name="sb", bufs=1))
        ps = ctx.enter_context(tc.tile_pool(name="ps", space="PSUM", bufs=1))

        # ---------------- constants ----------------
        m8t = consts.tile([128, CT, C], CDT, tag="m8t")
        wbt = consts.tile([128, CT, C + 16], CDT, tag="wbt")

        cblob = consts.tile([128, 20], F32, tag="cblob")
        nc.sync.dma_start(out=cblob, in_=cb_ext[:, :])
        b_sb = {}
        for bi, b in enumerate(("gnw", "gnb", "c0")):
            b_sb[b] = [cblob[:, bi * CT + ct:bi * CT + ct + 1]
                       for ct in range(CT)]
        gmat = cblob[:, 12:12 + GPT]

        ones16 = consts.tile([128, 2, 128], CDT, tag="ones16")
        nc.sync.dma_start(out=ones16, in_=ones_ext[:, :, :])
        gmt = consts.tile([GPT, 128], F32, tag="gmt")
        nc.sync.dma_start(out=gmt, in_=gmt_ext[:, :])
        eps_g = consts.tile([GPT, 1], F32, tag="eps_g")
        nc.vector.memset(eps_g, EPS)

        def warm_spin(k):
            warm = ps.tile([128, 128], F32, tag="small", bufs=2)
            for wi in range(k):
                if COMPUTE == "fp8":
                    nc.tensor.matmul(warm, ones16[:, 0:2, :], ones16[:, 0:2, :],
                                     start=(wi == 0), stop=(wi == k - 1),
                                     perf_mode=DR)
                else:
                    nc.tensor.matmul(warm, ones16[:, 0, :], ones16[:, 0, :],
                                     start=(wi == 0), stop=(wi == k - 1))

        # ---------------- per-sample pipeline phases ----------------
        st = [dict() for _ in range(S)]

        def phase_load(s):
            xt = sb.tile([128, CT, N], F32, name=f"x{s}", tag="x", bufs=2)
            for ct in range(CT):
                for q in range(2):
                    nc.sync.dma_start(out=xt[:, ct, q * 512:(q + 1) * 512],
                                      in_=x_ext[s, ct, :, q * 512:(q + 1) * 512])
            st[s]["x"] = xt

        def phase_gn_stats(s):
            # DVE/ACT only - no PE instructions, so this can be emitted
            # early without blocking the PE FIFO on slow stats.
            xt = st[s]["x"]
            stats3 = []
            for ct in range(CT):
                s3 = sb.tile([128, 3], F32, tag=f"s3_{ct}", bufs=2)
                if ct < 2:
                    # DVE path: bn_stats -> [mean, var], plus mean^2
                    st6 = sb.tile([128, 2, 6], F32, tag="st6", bufs=4)
                    nc.vector.bn_stats(out=st6[:, 0, :], in_=xt[:, ct, 0:512])
                    nc.vector.bn_stats(out=st6[:, 1, :], in_=xt[:, ct, 512:1024])
                    nc.vector.bn_aggr(out=s3[:, 0:2], in_=st6)
                    nc.vector.tensor_mul(out=s3[:, 2:3], in0=s3[:, 0:1],
                                         in1=s3[:, 0:1])
                else:
                    # ACT path: accum_out sums along the free axis.
                    # col0 = mean, col1 = E[x^2], col2 = 0. Downstream uses
                    # col1+col2 = E[x^2] = var + mean^2, same as the DVE path.
                    scr = sb.tile([128, N], CDT, tag="gnscr", bufs=2)
                    nc.scalar.activation(
                        out=scr, in_=xt[:, ct, :],
                        func=mybir.ActivationFunctionType.Copy,
                        scale=1.0 / N, accum_out=s3[:, 0:1])
                    nc.scalar.activation(
                        out=scr, in_=xt[:, ct, :],
                        func=mybir.ActivationFunctionType.Square,
                        scale=1.0 / float(np.sqrt(N)), accum_out=s3[:, 1:2])
                    nc.vector.memset(s3[:, 2:3], 0.0)
                stats3.append(s3)
            st[s]["stats3"] = stats3

        def phase_gn_ab(s):
            stats3 = st[s]["stats3"]
            ab = []
            for ct in range(CT):
                gp = ps.tile([GPT, 3], F32, tag="small", bufs=2)
                nc.tensor.matmul(gp, gmat, stats3[ct], start=True, stop=True)
                gs = sb.tile([GPT, 3], F32, tag="gs", bufs=4)
                nc.vector.tensor_copy(out=gs, in_=gp)
                # var_g = (E[var] + E[mean^2]) - (E[mean])^2
                m2 = sb.tile([GPT, 3], F32, tag="m2", bufs=4)
                nc.vector.tensor_add(out=m2[:, 1:2], in0=gs[:, 1:2],
                                     in1=gs[:, 2:3])
                nc.vector.tensor_mul(out=m2[:, 0:1], in0=gs[:, 0:1],
                                     in1=gs[:, 0:1])
                nc.vector.tensor_sub(out=m2[:, 2:3], in0=m2[:, 1:2],
                                     in1=m2[:, 0:1])
                # rsqrt(var+eps) = exp(-0.5*ln(var+eps)): Ln/Exp share one
                # ACT table set with Copy/Square/Identity (no table reloads)
                s2 = sb.tile([GPT, 2], F32, tag="s2", bufs=4)
                nc.scalar.activation(out=s2[:, 1:2], in_=m2[:, 2:3],
                                     func=mybir.ActivationFunctionType.Ln,
                                     bias=eps_g, scale=1.0)
                nc.scalar.activation(out=s2[:, 1:2], in_=s2[:, 1:2],
                                     func=mybir.ActivationFunctionType.Exp,
                                     scale=-0.5)
                nc.gpsimd.tensor_copy(out=s2[:, 0:1], in_=gs[:, 0:1])

                abp = ps.tile([128, 2], F32, tag="small", bufs=2)
                nc.tensor.matmul(abp, gmt, s2, start=True, stop=True)
                a_c = sb.tile([128, 1], F32, tag=f"a_{ct}", bufs=2)
                nc.vector.tensor_mul(out=a_c, in0=abp[:, 1:2],
                                     in1=b_sb["gnw"][ct])
                bneg = sb.tile([128, 1], F32, tag=f"bneg_{ct}", bufs=2)
                nc.vector.scalar_tensor_tensor(
                    out=bneg, in0=abp[:, 0:1], scalar=a_c,
                    in1=b_sb["gnb"][ct],
                    op0=mybir.AluOpType.mult, op1=mybir.AluOpType.subtract,
                )
                ab.append((a_c, bneg))
            st[s]["ab"] = ab

        def phase_h(s):
            xt = st[s]["x"]
            ab = st[s]["ab"]
            ht = sb.tile([128, CT, N], CDT, name=f"h{s}", tag="h", bufs=2)
            for ct in range(CT):
                a_c, bneg = ab[ct]
                if ct < 2:
                    nc.vector.tensor_scalar(
                        out=ht[:, ct, :], in0=xt[:, ct, :], scalar1=a_c,
                        scalar2=bneg,
                        op0=mybir.AluOpType.mult, op1=mybir.AluOpType.subtract,
                    )
                else:
                    # Pool can't write fp8; ACT computes a*x + (-bneg)
                    bpos = sb.tile([128, 1], F32, tag=f"bp_{ct}", bufs=2)
                    nc.gpsimd.tensor_scalar_mul(out=bpos, in0=bneg,
                                                scalar1=-1.0)
                    nc.scalar.activation(
                        out=ht[:, ct, :], in_=xt[:, ct, :],
                        func=mybir.ActivationFunctionType.Identity,
                        scale=a_c, bias=bpos,
                    )
            st[s]["h"] = ht

        def phase_g(s):
            # G = M8 @ h: stationary = m8t pair-slice, moving = h.
            # Both nch halves of one ot-block land in a [128, 1024] psum
            # (2 banks); a single 1024-wide evacuation halves instruction
            # overhead. Evacs alternate ACT/DVE.
            ht = st[s]["h"]
            gt = sb.tile([128, CT, N], CDT, name=f"g{s}", tag="g", bufs=2)
            for ot in range(CT):
                pp = ps.tile([128, N], F32, tag="mm2", bufs=3)
                for nch in range(NCH):
                    _mm_accum(
                        nc, pp[:, nch * NW:(nch + 1) * NW],
                        lambda j: m8t[:, j, ot * 128:(ot + 1) * 128],
                        lambda j: ht[:, j, nch * NW:(nch + 1) * NW],
                        CT)
                if ot % 2 == 0:
                    nc.scalar.copy(out=gt[:, ot, :], in_=pp)
                else:
                    nc.vector.tensor_copy(out=gt[:, ot, :], in_=pp)
            st[s]["g"] = gt

        def phase_vb(s):
            # Vb[m, o] = h_tile.T @ WB.T; z rides as moving column 512.
            # Two mt-blocks share one [128, 1024] psum -> single evac.
            ht = st[s]["h"]
            vt = sb.tile([128, MT, C], CDT, name=f"v{s}", tag="v", bufs=2)
            zps = ps.tile([128, MT], F32, tag="small", bufs=2)
            for mt0 in range(0, MT, 2):
                vp = ps.tile([128, N], F32, tag="mm2", bufs=3)
                for q in range(2):
                    mt = mt0 + q
                    _mm_accum(
                        nc, vp[:, q * C:(q + 1) * C],
                        lambda j: ht[:, j, mt * 128:(mt + 1) * 128],
                        lambda j: wbt[:, j, 0:C],
                        CT)
                    # z[m] column: same stationary, 1-col moving
                    _mm_accum(
                        nc, zps[:, mt:mt + 1],
                        lambda j: ht[:, j, mt * 128:(mt + 1) * 128],
                        lambda j: wbt[:, j, C:C + 1],
                        CT)
                nc.vector.tensor_copy(out=vt[:, mt0:mt0 + 2, :], in_=vp)
            # exp bias: z*ISC - SHIFT  (per-partition per-mt)
            bias_t = sb.tile([128, MT], F32, name=f"bias{s}", tag="bias",
                             bufs=2)
            nc.vector.tensor_scalar(
                out=bias_t, in0=zps, scalar1=ISC, scalar2=SHIFT,
                op0=mybir.AluOpType.mult, op1=mybir.AluOpType.subtract,
            )
            st[s]["v"], st[s]["bias"] = vt, bias_t

        def phase_st(s):
            # ST = G_tile.T @ h, mt-outer over the full pixel range; one
            # [128, 1024] psum and one 1024-wide exp per mt tile.
            ht, gt, bias_t = st[s]["h"], st[s]["g"], st[s]["bias"]
            et = sb.tile([128, MT, N], CDT, name=f"e{s}", tag="e", bufs=2)
            st[s]["e"] = et
            for mt in range(MT):
                sp = ps.tile([128, N], F32, tag="mm2", bufs=3)
                for nch in range(NCH):
                    _mm_accum(
                        nc, sp[:, nch * NW:(nch + 1) * NW],
                        lambda j: gt[:, j, mt * 128:(mt + 1) * 128],
                        lambda j: ht[:, j, nch * NW:(nch + 1) * NW],
                        CT)
                nc.scalar.activation(
                    out=et[:, mt, :], in_=sp,
                    func=mybir.ActivationFunctionType.Exp,
                    bias=bias_t[:, mt:mt + 1], scale=ISC,
                )

        def phase_r(s):
            # R = 1 / (SCL * colsum(E)) via ones(SCL) matmul + reciprocal
            et = st[s]["e"]
            rt = sb.tile([128, N], F32, name=f"r{s}", tag="r", bufs=2)
            st[s]["r"] = rt
            srp = ps.tile([128, N], F32, tag="mm2", bufs=3)
            steps = list(range(0, MT, KSTEP))
            for nch in range(NCH):
                for i, k in enumerate(steps):
                    nc.tensor.matmul(
                        srp[:, nch * NW:(nch + 1) * NW], ones16[:, 0:KSTEP, :],
                        et[:, k:k + KSTEP, nch * NW:(nch + 1) * NW],
                        start=(i == 0), stop=(i == len(steps) - 1),
                        perf_mode=DR)
            # 1/d = exp(-ln(d)) on ACT: both funcs sit in the same ACT
            # table set as Copy/Square/Identity, and DVE's reciprocal is
            # ~7.7ns/col (iterative) which would stall the OT evacuation.
            nc.scalar.activation(out=rt, in_=srp,
                                 func=mybir.ActivationFunctionType.Ln)
            nc.scalar.activation(out=rt, in_=rt,
                                 func=mybir.ActivationFunctionType.Exp,
                                 scale=-1.0)

        def phase_ot(s):
            # OT = Vb_tile.T @ E ; y = OT*R + x, in place over x.
            # DVE does the R multiply out of psum; the residual add
            # alternates Pool/DVE (Pool can't read psum, hence tmp).
            xt, vt, et, rt = st[s]["x"], st[s]["v"], st[s]["e"], st[s]["r"]
            steps = list(range(0, MT, KSTEP))
            for ct in range(CT):
                op_ = ps.tile([128, N], F32, tag="mm2", bufs=3)
                for nch in range(NCH):
                    for i, k in enumerate(steps):
                        nc.tensor.matmul(
                            op_[:, nch * NW:(nch + 1) * NW],
                            vt[:, k:k + KSTEP, ct * 128:(ct + 1) * 128],
                            et[:, k:k + KSTEP, nch * NW:(nch + 1) * NW],
                            start=(i == 0), stop=(i == len(steps) - 1),
                            perf_mode=DR)
                tmp = sb.tile([128, N], F32, tag="otmp", bufs=3)
                nc.vector.tensor_mul(out=tmp, in0=op_, in1=rt)
                # Pool is slow (~3.3us per 1024-wide add) but off the
                # critical path; the last tile's add goes to DVE so the
                # kernel tail isn't gated on Pool.
                eng = nc.vector if ct == CT - 1 else nc.gpsimd
                eng.tensor_add(out=xt[:, ct, :], in0=tmp, in1=xt[:, ct, :])
                nc.sync.dma_start(out=out_ext[s, ct, :, :], in_=xt[:, ct, :])

        # ---------------- emission ----------------
        # Front is pipelined: s0's GN/h feed G/Vb(s0) on the PE while
        # s1's GN runs on DVE/ACT; ST/R/OT phases then alternate samples
        # so one sample's exp chain hides behind the other's matmuls.
        warm_spin(8)
        phase_load(0)
        nc.sync.dma_start(out=m8t, in_=m8_ext[:, :, :])
        phase_load(1)
        nc.sync.dma_start(out=wbt, in_=wb_ext[:, :, :])

        phase_gn_stats(0)
        phase_gn_ab(0)
        phase_h(0)
        phase_gn_stats(1)    # DVE/ACT work only; PE proceeds to G(0)
        phase_g(0)
        phase_vb(0)
        phase_gn_ab(1)
        phase_h(1)
        phase_st(0)          # exp(s0) drains behind G(1)/Vb(1) matmuls
        phase_g(1)
        phase_vb(1)
        phase_r(0)
        phase_ot(0)
        phase_st(1)
        phase_r(1)
        phase_ot(1)


_CACHE = {}


def make_in_maps(inputs):
    """Host-side sharding/layout prep shared by kernel() and the test/sim
    harnesses."""
    import ml_dtypes

    x = np.asarray(inputs["x"], dtype=np.float32)
    assert x.shape == (B, C, H, W)

    wdt = ml_dtypes.float8_e4m3fn if COMPUTE == "fp8" else ml_dtypes.bfloat16

    def q(a):
        return np.clip(a * SCL, -240.0, 240.0).astype(wdt)

    wq = np.asarray(inputs["wq"], dtype=np.float64)
    wk = np.asarray(inputs["wk"], dtype=np.float64)
    wv = np.asarray(inputs["wv"], dtype=np.float64)
    wo = np.asarray(inputs["wo"], dtype=np.float64)
    bq = np.asarray(inputs["bq"], dtype=np.float64)
    bv = np.asarray(inputs["bv"], dtype=np.float64)
    bo = np.asarray(inputs["bo"], dtype=np.float64)

    M8 = wq.T @ wk                 # [c, c']
    WB = wo @ wv                   # [o, c]
    u = wk.T @ bq                  # [c'] per-key softmax bias vector
    c0 = (wo @ bv + bo).astype(np.float32)

    # m8t[p, j, o] = M8[o, j*128+p]  (stationary tiles of M8^T)
    m8t = q(np.ascontiguousarray(
        M8.T.reshape(CT, 128, C).transpose(1, 0, 2)))
    # wbt[p, j, 0:512] = WB[o, j*128+p]^T tiles; col 512 = u; pad to 528
    wbt = np.zeros((128, CT, C + 16), dtype=np.float64)
    wbt[:, :, 0:C] = WB.T.reshape(CT, 128, C).transpose(1, 0, 2)
    wbt[:, :, C] = u.reshape(CT, 128).T
    wbt = q(wbt)

    gmat = np.zeros((128, GPT), dtype=np.float32)
    gmt = np.zeros((GPT, 128), dtype=np.float32)
    for g in range(GPT):
        gmat[g * GSIZE:(g + 1) * GSIZE, g] = 1.0 / GSIZE
        gmt[g, g * GSIZE:(g + 1) * GSIZE] = 1.0
    cblob = np.zeros((128, 20), dtype=np.float32)
    for bi, arr in enumerate((inputs["gn_weight"], inputs["gn_bias"], c0)):
        cblob[:, bi * CT:(bi + 1) * CT] = np.asarray(
            arr, dtype=np.float32).reshape(CT, 128).T
    cblob[:, 12:12 + GPT] = gmat

    base = {
        "m8t": m8t, "wbt8": wbt, "cblob": cblob, "gmt": gmt,
        "ones16": np.full((128, 2, 128), SCL, dtype=wdt),
    }
    xr = x.reshape(NCORES, S, CT, 128, N)
    return [dict(base, x=np.ascontiguousarray(xr[i])) for i in range(NCORES)]


def kernel(**inputs):
    if "nc" not in _CACHE:
        _CACHE["nc"] = build_nc()
    nc = _CACHE["nc"]

    in_maps = make_in_maps(inputs)
    res = run_bass_kernel_spmd(nc, in_maps, core_ids=list(range(NCORES)))

    out = np.empty((NCORES, S, CT, 128, N), dtype=np.float32)
    for i in range(NCORES):
        out[i] = res.results[i]["out"]
    out = out.reshape(B, C, H, W)
    # c0 = Wo bv + bo enters the output as a per-channel constant
    # (softmax rows sum to 1); zero for zero biases.
    c0 = (np.asarray(inputs["wo"], dtype=np.float64)
          @ np.asarray(inputs["bv"], dtype=np.float64)
          + np.asarray(inputs["bo"], dtype=np.float64)).astype(np.float32)
    if np.any(c0):
        out += c0[None, :, None, None]
    return out


# revision 24
# speedup vs baseline: 1.2334x; 1.2334x over previous
"""GroupNorm + single-head self-attention block (B=16, C=512, H=W=32) on 8
TRN2 NeuronCores.

Sharding: pure data-parallel over batch - 2 samples per core, no collectives.

Algebraic restructure vs the straightforward q/k/v pipeline (exact, done on
the host in f64):

  scores  = (Wq h)^T (Wk h) = h^T M h        with M  = Wq^T Wk
  o       = attn @ (V Wo^T)                  with WB = Wo @ Wv  (attn weights
            are scalars, so the output projection commutes into V)

so the device only runs TWO projection matmuls (G = M h and Vb = WB h)
instead of four (q, k, v, o2): 25% fewer PE columns and ~30% fewer PSUM
evacuations.  Biases stay exact:
  - bv, bo enter as c0 = Wo bv + bo, a per-channel constant added at the
    final (channel-major) evacuation;
  - bq, bk survive softmax only through the per-key term z[m] = (Wk^T bq) .
    h[:, m]; z is computed as one extra moving column riding the Vb matmul
    (whose output is pixel-major, so z lands per-partition) and folded into
    the exp bias. Per-query terms cancel in softmax.

Matmuls run in fp8-e4m3 with perf_mode=DoubleRow (2 contraction slices per
pass), weights pre-scaled x16 into the e4m3 sweet spot, exp shifted by -1.5
so E stays far below the TRN fp8 max of 240. fp32 PSUM accumulation.
Numpy-prototyped rel err vs the fp32 reference: 4.3e-3 (budget 2e-2).

Per-sample dataflow (C=512 channels, N=1024 pixels), channels-on-partitions:

  x    [128, CT=4, N]   f32
  GN:  bn_stats/bn_aggr per channel, group (16-ch) aggregation via a tiny
       matmul against a group-indicator matrix; h = a_c*x - b_c -> fp8
  G    [128, CT, N] = (M.T tiles).T @ h        (DR pairs over c')
  Vb   [128, MT=8, C] = h_tile.T @ WB.T tiles  (pixel-major; z column rides)
  ST   [m, n] = G_tile.T @ h                   (scores, transposed layout)
  E    = exp(ST*ISC + bias)  -> fp8            (bias = z/sqrt(C) - SHIFT)
  R    = 1/(16.T ones @ E)                     (softmax denominators, PE)
  OT   [c, n] = Vb_tile.T @ E                  (un-normalized attn output)
  y    = (OT*R + c0) + x                       (DVE mul in psum, Pool stt)
"""

import numpy as np

import concourse.bass as bass
import concourse.mybir as mybir
from concourse import tile
from concourse.bass_utils import run_bass_kernel_spmd


def _install_drain_patch():
    """This walrus build rejects Drain instructions carrying more than one
    semaphore wait (setupSyncWait<CTRL_NO_STRUCT>). Split the TileContext
    tail drain's waits across a chain of single-wait drains."""
    import concourse.tile as tile_mod
    from concourse.vector_clock import ScopedClock

    if getattr(tile_mod.TileContext, "_drain_patch_installed", False):
        return

    def _patched(self, tick_clock, wait_clock):
        nc = self.nc
        drain_inst = nc.sync.drain()
        wait_clock.add_sem_waits(
            drain_inst.ins, ScopedClock({None: tick_clock.global_clock})
        )
        si = drain_inst.ins.sync_info
        waits = list(si.on_wait or []) if si is not None else []
        if len(waits) > 1:
            si.on_wait = waits[:1]
            for w in waits[1:]:
                extra = nc.sync.drain()
                extra.ins.sync_info = mybir.SyncInfo(on_wait=[w], on_update=[])

        nc.all_engine_barrier()
        assert self.sems is not None
        popped = nc._tile_sem_poison_stack.pop()
        assert popped is self._sem_poison
        nc.clear_and_free_semaphores(list(self.sems.allocated().values()))
        nc.all_engine_barrier()

    tile_mod.TileContext._drain_and_barrier = _patched
    tile_mod.TileContext._drain_patch_installed = True


_install_drain_patch()

F32 = mybir.dt.float32
BF16 = mybir.dt.bfloat16
FP8 = mybir.dt.float8e4

B, C, H, W = 16, 512, 32, 32
N = H * W                      # 1024 pixels
NCORES = 8
S = B // NCORES                # samples per core
CT = C // 128                  # 4 channel tiles
MT = N // 128                  # 8 pixel tiles
NW = 512                       # matmul output chunk (PSUM bank = 512 f32)
NCH = N // NW                  # 2 chunks
GROUPS = 32
GSIZE = C // GROUPS            # 16 channels per group
GPT = 128 // GSIZE             # 8 groups per channel tile
EPS = 1e-5

COMPUTE = "fp8"                # "fp8" (DoubleRow) | "bf16"

CDT = {"fp8": FP8, "bf16": BF16}[COMPUTE]
SCL = 16.0 if COMPUTE == "fp8" else 1.0
SHIFT = 1.5
ISC = 1.0 / (SCL * float(np.sqrt(C)))
DR = mybir.MatmulPerfMode.DoubleRow if COMPUTE == "fp8" else None
KSTEP = 2 if COMPUTE == "fp8" else 1   # contraction subtiles per matmul


def _split_waits(nc, maxw=1):
    """This walrus build caps the number of sync waits an instruction can
    carry (varies by instruction class; Drain and Matmult/LDWEIGHTS observed
    failing). Hoist excess waits onto standalone EventSemaphore instructions
    inserted just before, on the same engine."""
    cnt = 0
    for f in nc.m.functions:
        for bb in f.blocks:
            insts = list(bb.instructions)
            out = []
            changed = False
            for inst in insts:
                si = inst.sync_info
                waits = list(si.on_wait) if (si is not None and si.on_wait) else []
                if len(waits) > maxw:
                    for w in waits[:-maxw]:
                        ev = mybir.InstEventSemaphore(
                            name=f"waitsplit_{cnt}", ins=[], outs=[])
                        cnt += 1
                        ev.engine = inst.engine
                        ev.sync_info = mybir.SyncInfo(on_wait=[w], on_update=[])
                        out.append(ev)
                    si.on_wait = waits[-maxw:]
                    changed = True
                out.append(inst)
            if changed:
                _replace_block_instructions(bb, out)
    return cnt


def _replace_block_instructions(bb, insts):
    try:
        bb.instructions = insts
        return
    except Exception:
        pass
    try:
        bb.instructions.clear()
        for i in insts:
            bb.instructions.append(i)
        return
    except Exception:
        pass
    raise RuntimeError("cannot rewrite block instructions")


def build_nc(split_waits=True):
    nc = bass.Bass(target_bir_lowering=False)

    x_ext = nc.declare_dram_parameter("x", [S, CT, 128, N], F32, isOutput=False)
    m8_ext = nc.declare_dram_parameter("m8t", [128, CT, C], CDT, isOutput=False)
    # wbt carries WB.T in cols 0:512 and the z-vector u in col 512 (+pad)
    wb_ext = nc.declare_dram_parameter("wbt8", [128, CT, C + 16], CDT,
                                       isOutput=False)
    # cblob columns: gnw[4] gnb[4] c0[4] gmat[8] -> [128, 20] f32
    cb_ext = nc.declare_dram_parameter("cblob", [128, 20], F32, isOutput=False)
    gmt_ext = nc.declare_dram_parameter("gmt", [GPT, 128], F32, isOutput=False)
    ones_ext = nc.declare_dram_parameter("ones16", [128, 2, 128], CDT,
                                         isOutput=False)
    out_ext = nc.declare_dram_parameter("out", [S, CT, 128, N], F32,
                                        isOutput=True)

    with tile.TileContext(nc) as tc:
        _body(nc, tc, x_ext, m8_ext, wb_ext, cb_ext, gmt_ext, ones_ext,
              out_ext)
    if split_waits:
        _split_waits(nc)
    return nc


def _mm_accum(nc, psum, lhsT3, rhs3, kparts):
    """Accumulating matmul over contraction subtiles. lhsT3/rhs3 are
    callables j -> AP: for fp8 they get slice(j, j+2) (DoubleRow pair),
    for bf16 slice(j, j+1)."""
    steps = list(range(0, kparts, KSTEP))
    for i, j in enumerate(steps):
        nc.tensor.matmul(
            psum,
            lhsT3(slice(j, j + KSTEP)),
            rhs3(slice(j, j + KSTEP)),
            start=(i == 0), stop=(i == len(steps) - 1),
            perf_mode=DR,
        )


def _body(nc, tc, x_ext, m8_ext, wb_ext, cb_ext, gmt_ext, ones_ext, out_ext):
    import contextlib

    ctx = contextlib.ExitStack()
    with ctx:
        consts = ctx.enter_context(tc.tile_pool(name="consts", bufs=1))
        sb = ctx.enter_context(tc.tile_pool(name="sb", bufs=1))
        ps = ctx.enter_context(tc.tile_pool(name="ps", space="PSUM", bufs=1))

        # ---------------- constants ----------------
        m8t = consts.tile([128, CT, C], CDT, tag="m8t")
        wbt = consts.tile([128, CT, C + 16], CDT, tag="wbt")

        cblob = consts.tile([128, 20], F32, tag="cblob")
        nc.sync.dma_start(out=cblob, in_=cb_ext[:, :])
        b_sb = {}
        for bi, b in enumerate(("gnw", "gnb", "c0")):
            b_sb[b] = [cblob[:, bi * CT + ct:bi * CT + ct + 1]
                       for ct in range(CT)]
        gmat = cblob[:, 12:12 + GPT]

        ones16 = consts.tile([128, 2, 128], CDT, tag="ones16")
        nc.sync.dma_start(out=ones16, in_=ones_ext[:, :, :])
        gmt = consts.tile([GPT, 128], F32, tag="gmt")
        nc.sync.dma_start(out=gmt, in_=gmt_ext[:, :])
        eps_g = consts.tile([GPT, 1], F32, tag="eps_g")
        nc.vector.memset(eps_g, EPS)

        def warm_spin(k):
            warm = ps.tile([128, 128], F32, tag="small", bufs=2)
            for wi in range(k):
                if COMPUTE == "fp8":
                    nc.tensor.matmul(warm, ones16[:, 0:2, :], ones16[:, 0:2, :],
                                     start=(wi == 0), stop=(wi == k - 1),
                                     perf_mode=DR)
                else:
                    nc.tensor.matmul(warm, ones16[:, 0, :], ones16[:, 0, :],
                                     start=(wi == 0), stop=(wi == k - 1))

        # ---------------- per-sample pipeline phases ----------------
        st = [dict() for _ in range(S)]

        def phase_load(s):
            xt = sb.tile([128, CT, N], F32, name=f"x{s}", tag="x", bufs=2)
            # ct2,3 first: their (slower, serial) ACT stats start earliest
            for ct in (2, 3, 0, 1):
                for q in range(2):
                    nc.sync.dma_start(out=xt[:, ct, q * 512:(q + 1) * 512],
                                      in_=x_ext[s, ct, :, q * 512:(q + 1) * 512])
            st[s]["x"] = xt

        def phase_gn_stats(s):
            # DVE/ACT only - no PE instructions, so this can be emitted
            # early without blocking the PE FIFO on slow stats.
            xt = st[s]["x"]
            stats3 = []
            for ct in range(CT):
                s3 = sb.tile([128, 3], F32, tag=f"s3_{ct}", bufs=2)
                if ct < 2:
                    # DVE path: bn_stats -> [mean, var], plus mean^2
                    st6 = sb.tile([128, 2, 6], F32, tag="st6", bufs=4)
                    nc.vector.bn_stats(out=st6[:, 0, :], in_=xt[:, ct, 0:512])
                    nc.vector.bn_stats(out=st6[:, 1, :], in_=xt[:, ct, 512:1024])
                    nc.vector.bn_aggr(out=s3[:, 0:2], in_=st6)
                    nc.vector.tensor_mul(out=s3[:, 2:3], in0=s3[:, 0:1],
                                         in1=s3[:, 0:1])
                else:
                    # ACT path: accum_out sums along the free axis.
                    # col0 = mean, col1 = E[x^2], col2 = 0. Downstream uses
                    # col1+col2 = E[x^2] = var + mean^2, same as the DVE path.
                    scr = sb.tile([128, N], CDT, tag="gnscr", bufs=2)
                    nc.scalar.activation(
                        out=scr, in_=xt[:, ct, :],
                        func=mybir.ActivationFunctionType.Copy,
                        scale=1.0 / N, accum_out=s3[:, 0:1])
                    nc.scalar.activation(
                        out=scr, in_=xt[:, ct, :],
                        func=mybir.ActivationFunctionType.Square,
                        scale=1.0 / float(np.sqrt(N)), accum_out=s3[:, 1:2])
                    nc.vector.memset(s3[:, 2:3], 0.0)
                stats3.append(s3)
            st[s]["stats3"] = stats3

        def phase_gn_ab(s):
            stats3 = st[s]["stats3"]
            ab = []
            for ct in range(CT):
                gp = ps.tile([GPT, 3], F32, tag="small", bufs=2)
                nc.tensor.matmul(gp, gmat, stats3[ct], start=True, stop=True)
                gs = sb.tile([GPT, 3], F32, tag="gs", bufs=4)
                nc.vector.tensor_copy(out=gs, in_=gp)
                # var_g = (E[var] + E[mean^2]) - (E[mean])^2
                m2 = sb.tile([GPT, 3], F32, tag="m2", bufs=4)
                nc.vector.tensor_add(out=m2[:, 1:2], in0=gs[:, 1:2],
                                     in1=gs[:, 2:3])
                nc.vector.tensor_mul(out=m2[:, 0:1], in0=gs[:, 0:1],
                                     in1=gs[:, 0:1])
                nc.vector.tensor_sub(out=m2[:, 2:3], in0=m2[:, 1:2],
                                     in1=m2[:, 0:1])
                # rsqrt(var+eps) = exp(-0.5*ln(var+eps)): Ln/Exp share one
                # ACT table set with Copy/Square/Identity (no table reloads)
                s2 = sb.tile([GPT, 2], F32, tag="s2", bufs=4)
                nc.scalar.activation(out=s2[:, 1:2], in_=m2[:, 2:3],
                                     func=mybir.ActivationFunctionType.Ln,
                                     bias=eps_g, scale=1.0)
                nc.scalar.activation(out=s2[:, 1:2], in_=s2[:, 1:2],
                                     func=mybir.ActivationFunctionType.Exp,
                                     scale=-0.5)
                nc.gpsimd.tensor_copy(out=s2[:, 0:1], in_=gs[:, 0:1])

                abp = ps.tile([128, 2], F32, tag="small", bufs=2)
                nc.tensor.matmul(abp, gmt, s2, start=True, stop=True)
                a_c = sb.tile([128, 1], F32, tag=f"a_{ct}", bufs=2)
                nc.vector.tensor_mul(out=a_c, in0=abp[:, 1:2],
                                     in1=b_sb["gnw"][ct])
                bneg = sb.tile([128, 1], F32, tag=f"bneg_{ct}", bufs=2)
                nc.vector.scalar_tensor_tensor(
                    out=bneg, in0=abp[:, 0:1], scalar=a_c,
                    in1=b_sb["gnb"][ct],
                    op0=mybir.AluOpType.mult, op1=mybir.AluOpType.subtract,
                )
                ab.append((a_c, bneg))
            st[s]["ab"] = ab

        def phase_h(s):
            xt = st[s]["x"]
            ab = st[s]["ab"]
            ht = sb.tile([128, CT, N], CDT, name=f"h{s}", tag="h", bufs=2)
            for ct in range(CT):
                a_c, bneg = ab[ct]
                # s1's h entirely on DVE so ACT's queue is clear when the
                # s0 exp chain (latency-critical for R) becomes runnable
                if ct < 2 or s == 1:
                    nc.vector.tensor_scalar(
                        out=ht[:, ct, :], in0=xt[:, ct, :], scalar1=a_c,
                        scalar2=bneg,
                        op0=mybir.AluOpType.mult, op1=mybir.AluOpType.subtract,
                    )
                else:
                    # Pool can't write fp8; ACT computes a*x + (-bneg)
                    bpos = sb.tile([128, 1], F32, tag=f"bp_{ct}", bufs=2)
                    nc.gpsimd.tensor_scalar_mul(out=bpos, in0=bneg,
                                                scalar1=-1.0)
                    nc.scalar.activation(
                        out=ht[:, ct, :], in_=xt[:, ct, :],
                        func=mybir.ActivationFunctionType.Identity,
                        scale=a_c, bias=bpos,
                    )
            st[s]["h"] = ht

        def phase_g(s):
            # G = M8 @ h: stationary = m8t pair-slice, moving = h.
            # Both nch halves of one ot-block land in a [128, 1024] psum
            # (2 banks); a single 1024-wide evacuation halves instruction
            # overhead. Evacs alternate ACT/DVE.
            ht = st[s]["h"]
            gt = sb.tile([128, CT, N], CDT, name=f"g{s}", tag="g", bufs=2)
            for ot in range(CT):
                pp = ps.tile([128, N], F32, tag="mm2", bufs=3)
                for nch in range(NCH):
                    _mm_accum(
                        nc, pp[:, nch * NW:(nch + 1) * NW],
                        lambda j: m8t[:, j, ot * 128:(ot + 1) * 128],
                        lambda j: ht[:, j, nch * NW:(nch + 1) * NW],
                        CT)
                # all G evacs on DVE: ACT must stay clear for the exp chain
                nc.vector.tensor_copy(out=gt[:, ot, :], in_=pp)
            st[s]["g"] = gt

        def phase_vb(s):
            # Vb[m, o] = h_tile.T @ WB.T; z rides as moving column 512.
            # Two mt-blocks share one [128, 1024] psum -> single evac.
            ht = st[s]["h"]
            vt = sb.tile([128, MT, C], CDT, name=f"v{s}", tag="v", bufs=2)
            zps = ps.tile([128, MT], F32, tag="small", bufs=2)
            for mt0 in range(0, MT, 2):
                vp = ps.tile([128, N], F32, tag="mm2", bufs=3)
                for q in range(2):
                    mt = mt0 + q
                    _mm_accum(
                        nc, vp[:, q * C:(q + 1) * C],
                        lambda j: ht[:, j, mt * 128:(mt + 1) * 128],
                        lambda j: wbt[:, j, 0:C],
                        CT)
                    # z[m] column: same stationary, 1-col moving
                    _mm_accum(
                        nc, zps[:, mt:mt + 1],
                        lambda j: ht[:, j, mt * 128:(mt + 1) * 128],
                        lambda j: wbt[:, j, C:C + 1],
                        CT)
                nc.vector.tensor_copy(out=vt[:, mt0:mt0 + 2, :], in_=vp)
            # exp bias: z*ISC - SHIFT  (per-partition per-mt)
            bias_t = sb.tile([128, MT], F32, name=f"bias{s}", tag="bias",
                             bufs=2)
            nc.vector.tensor_scalar(
                out=bias_t, in0=zps, scalar1=ISC, scalar2=SHIFT,
                op0=mybir.AluOpType.mult, op1=mybir.AluOpType.subtract,
            )
            st[s]["v"], st[s]["bias"] = vt, bias_t

        def phase_st(s):
            # ST = G_tile.T @ h, mt-outer over the full pixel range; one
            # [128, 1024] psum and one 1024-wide exp per mt tile.
            ht, gt, bias_t = st[s]["h"], st[s]["g"], st[s]["bias"]
            et = sb.tile([128, MT, N], CDT, name=f"e{s}", tag="e", bufs=2)
            st[s]["e"] = et
            for mt in range(MT):
                sp = ps.tile([128, N], F32, tag="mm2", bufs=3)
                for nch in range(NCH):
                    _mm_accum(
                        nc, sp[:, nch * NW:(nch + 1) * NW],
                        lambda j: gt[:, j, mt * 128:(mt + 1) * 128],
                        lambda j: ht[:, j, nch * NW:(nch + 1) * NW],
                        CT)
                nc.scalar.activation(
                    out=et[:, mt, :], in_=sp,
                    func=mybir.ActivationFunctionType.Exp,
                    bias=bias_t[:, mt:mt + 1], scale=ISC,
                )

        def phase_r(s):
            # R = 1 / (SCL * colsum(E)) via ones(SCL) matmul + reciprocal
            et = st[s]["e"]
            rt = sb.tile([128, N], F32, name=f"r{s}", tag="r", bufs=2)
            st[s]["r"] = rt
            srp = ps.tile([128, N], F32, tag="mm2", bufs=3)
            steps = list(range(0, MT, KSTEP))
            for nch in range(NCH):
                for i, k in enumerate(steps):
                    nc.tensor.matmul(
                        srp[:, nch * NW:(nch + 1) * NW], ones16[:, 0:KSTEP, :],
                        et[:, k:k + KSTEP, nch * NW:(nch + 1) * NW],
                        start=(i == 0), stop=(i == len(steps) - 1),
                        perf_mode=DR)
            # 1/d = exp(-ln(d)) on ACT: both funcs sit in the same ACT
            # table set as Copy/Square/Identity, and DVE's reciprocal is
            # ~7.7ns/col (iterative) which would stall the OT evacuation.
            nc.scalar.activation(out=rt, in_=srp,
                                 func=mybir.ActivationFunctionType.Ln)
            nc.scalar.activation(out=rt, in_=rt,
                                 func=mybir.ActivationFunctionType.Exp,
                                 scale=-1.0)

        def phase_ot(s):
            # OT = Vb_tile.T @ E ; y = OT*R + x, in place over x.
            # DVE does the R multiply out of psum; the residual add
            # alternates Pool/DVE (Pool can't read psum, hence tmp).
            xt, vt, et, rt = st[s]["x"], st[s]["v"], st[s]["e"], st[s]["r"]
            steps = list(range(0, MT, KSTEP))
            for ct in range(CT):
                op_ = ps.tile([128, N], F32, tag="mm2", bufs=3)
                for nch in range(NCH):
                    for i, k in enumerate(steps):
                        nc.tensor.matmul(
                            op_[:, nch * NW:(nch + 1) * NW],
                            vt[:, k:k + KSTEP, ct * 128:(ct + 1) * 128],
                            et[:, k:k + KSTEP, nch * NW:(nch + 1) * NW],
                            start=(i == 0), stop=(i == len(steps) - 1),
                            perf_mode=DR)
                tmp = sb.tile([128, N], F32, tag="otmp", bufs=3)
                nc.vector.tensor_mul(out=tmp, in0=op_, in1=rt)
                # Pool is slow (~3.3us per 1024-wide add) but off the
                # critical path; the last tile's add goes to DVE so the
                # kernel tail isn't gated on Pool.
                eng = nc.vector if ct == CT - 1 else nc.gpsimd
                eng.tensor_add(out=xt[:, ct, :], in0=tmp, in1=xt[:, ct, :])
                nc.sync.dma_start(out=out_ext[s, ct, :, :], in_=xt[:, ct, :])

        # ---------------- emission ----------------
        # Front is pipelined: s0's GN/h feed G/Vb(s0) on the PE while
        # s1's GN runs on DVE/ACT; ST/R/OT phases then alternate samples
        # so one sample's exp chain hides behind the other's matmuls.
        warm_spin(8)
        phase_load(0)
        nc.sync.dma_start(out=m8t, in_=m8_ext[:, :, :])
        phase_load(1)
        nc.sync.dma_start(out=wbt, in_=wb_ext[:, :, :])

        phase_gn_stats(0)
        phase_gn_ab(0)
        phase_h(0)
        warm_spin(8)
        phase_gn_stats(1)
        phase_gn_ab(1)
        phase_g(0)
        phase_vb(0)
        phase_h(1)
        phase_st(0)          # exp(s0) drains behind G(1)/Vb(1) matmuls
        phase_g(1)
        phase_vb(1)
        phase_r(0)
        phase_ot(0)
        phase_st(1)
        phase_r(1)
        phase_ot(1)


_CACHE = {}


def make_in_maps(inputs):
    """Host-side sharding/layout prep shared by kernel() and the test/sim
    harnesses."""
    import ml_dtypes

    x = np.asarray(inputs["x"], dtype=np.float32)
    assert x.shape == (B, C, H, W)

    wdt = ml_dtypes.float8_e4m3fn if COMPUTE == "fp8" else ml_dtypes.bfloat16

    def q(a):
        return np.clip(a * SCL, -240.0, 240.0).astype(wdt)

    wq = np.asarray(inputs["wq"], dtype=np.float64)
    wk = np.asarray(inputs["wk"], dtype=np.float64)
    wv = np.asarray(inputs["wv"], dtype=np.float64)
    wo = np.asarray(inputs["wo"], dtype=np.float64)
    bq = np.asarray(inputs["bq"], dtype=np.float64)
    bv = np.asarray(inputs["bv"], dtype=np.float64)
    bo = np.asarray(inputs["bo"], dtype=np.float64)

    M8 = wq.T @ wk                 # [c, c']
    WB = wo @ wv                   # [o, c]
    u = wk.T @ bq                  # [c'] per-key softmax bias vector
    c0 = (wo @ bv + bo).astype(np.float32)

    # m8t[p, j, o] = M8[o, j*128+p]  (stationary tiles of M8^T)
    m8t = q(np.ascontiguousarray(
        M8.T.reshape(CT, 128, C).transpose(1, 0, 2)))
    # wbt[p, j, 0:512] = WB[o, j*128+p]^T tiles; col 512 = u; pad to 528
    wbt = np.zeros((128, CT, C + 16), dtype=np.float64)
    wbt[:, :, 0:C] = WB.T.reshape(CT, 128, C).transpose(1, 0, 2)
    wbt[:, :, C] = u.reshape(CT, 128).T
    wbt = q(wbt)

    gmat = np.zeros((128, GPT), dtype=np.float32)
    gmt = np.zeros((GPT, 128), dtype=np.float32)
    for g in range(GPT):
        gmat[g * GSIZE:(g + 1) * GSIZE, g] = 1.0 / GSIZE
        gmt[g, g * GSIZE:(g + 1) * GSIZE] = 1.0
    cblob = np.zeros((128, 20), dtype=np.float32)
    for bi, arr in enumerate((inputs["gn_weight"], inputs["gn_bias"], c0)):
        cblob[:, bi * CT:(bi + 1) * CT] = np.asarray(
            arr, dtype=np.float32).reshape(CT, 128).T
    cblob[:, 12:12 + GPT] = gmat

    base = {
        "m8t": m8t, "wbt8": wbt, "cblob": cblob, "gmt": gmt,
        "ones16": np.full((128, 2, 128), SCL, dtype=wdt),
    }
    xr = x.reshape(NCORES, S, CT, 128, N)
    return [dict(base, x=np.ascontiguousarray(xr[i])) for i in range(NCORES)]


def kernel(**inputs):
    if "nc" not in _CACHE:
        _CACHE["nc"] = build_nc()
    nc = _CACHE["nc"]

    in_maps = make_in_maps(inputs)
    res = run_bass_kernel_spmd(nc, in_maps, core_ids=list(range(NCORES)))

    out = np.empty((NCORES, S, CT, 128, N), dtype=np.float32)
    for i in range(NCORES):
        out[i] = res.results[i]["out"]
    out = out.reshape(B, C, H, W)
    # c0 = Wo bv + bo enters the output as a per-channel constant
    # (softmax rows sum to 1); zero for zero biases.
    c0 = (np.asarray(inputs["wo"], dtype=np.float64)
          @ np.asarray(inputs["bv"], dtype=np.float64)
          + np.asarray(inputs["bo"], dtype=np.float64)).astype(np.float32)
    if np.any(c0):
        out += c0[None, :, None, None]
    return out
